# revision 38
# baseline (speedup 1.0000x reference)
"""Trainium2 Bass kernel for nn_BEMBFlex (within-category log-softmax utility model).

Sharding: items dealt by category across the 8 cores (categories rank-sorted
by size, rank % 8 -> shard), so one SPMD program serves all cores. Each core
computes util for all 1024 sessions over its ~1/8 of the items, then the
within-category log-softmax locally (categories never span shards).

v2 pipeline (per 128-session chunk, per column block):
  TensorE:  psum = [th|ze] @ [alphaT; item_obsT]  (+ rank-1 matmul folding
            the per-item (lambda - SHIFT) row, so PSUM holds u1 directly);
            512-col PSUM pieces in a ring of 4x1024 keep the PE stream dense
  ScalarE:  ex_bf16 = Exp(psum)            (the only PSUM reader; 1 ACT pass)
  VectorE:  segment sums via bf16 halving tree (2x mode) + tensor_reduce
  VectorE:  lsc = bitcast_i32(s) * ln2/2^23       (log2 bit-hack, no ACT Ln)
  VectorE:  out_bf16 = (bitcast_i16(ex) * ln2/2^7) - lsc_bcast  (fused STT:
            ln(ex) via the same bit-hack; the (127+sigma)*ln2 biases cancel
            between the two hacks). GpSimd deliberately unused: it shares an
            SBUF port with the DVE, and offloading finals there slowed the
            DVE 2-port ops ~3x (measured).
  DMA out per block (bf16, largest blocks first), host de-permutes + casts.
"""

import sys

for _p in ("/opt/trn_rl_repo",):
    if _p not in sys.path:
        sys.path.insert(0, _p)

import ml_dtypes
import numpy as np

import concourse.bass as bass
import concourse.tile as tile
from concourse import bacc, bass_utils, mybir

NUM_USERS = 100000
NUM_ITEMS = 25000
NUM_CATS = 500
LATENT = 64
BATCH = 1024
NCORES = 8
P = 128
NCHUNKS = BATCH // P
BLOCK_COLS = 2048
PAD_NEG = -1.0e30
SHIFT = 18.0
LN2 = float(np.log(2.0))
K16 = LN2 / (1 << 7)       # bf16-bits  -> ln scale
K32 = LN2 / (1 << 23)      # fp32-bits  -> ln scale

F32 = mybir.dt.float32
BF16 = mybir.dt.bfloat16
I16 = mybir.dt.int16
I32 = mybir.dt.int32

_nc_cache = {}


# ----------------------------------------------------------------------------
# Host-side layout
# ----------------------------------------------------------------------------

def _layout(cat_sizes):
    """Slot/block layout shared by all 8 shards.

    Categories sorted by size desc; slot i holds category ranks [8i, 8i+8)
    (one per shard). Slot width L_i = largest size in the group rounded up
    to a multiple of 8 (so the halving tree stays 2x-mode friendly). Blocks
    greedily group consecutive slots under a uniform L.
    """
    order = np.argsort(-cat_sizes, kind="stable")
    order = order[cat_sizes[order] > 0]
    ncats = len(order)
    nslots = -(-ncats // NCORES)
    slot_L = np.empty(nslots, np.int64)
    for i in range(nslots):
        mx = int(cat_sizes[order[i * NCORES]])
        slot_L[i] = max(8, ((mx + 7) // 8) * 8)
    blocks = []  # (col0, g, L, slot0)
    col = 0
    i = 0
    while i < nslots:
        Lb = int(slot_L[i])
        g = 1
        sm = Lb
        # grow while under the col cap AND padding waste stays under 10%
        while (
            i + g < nslots
            and (g + 1) * Lb <= BLOCK_COLS
            and (g + 1) * Lb - (sm + slot_L[i + g]) <= 0.10 * (g + 1) * Lb
        ):
            sm += slot_L[i + g]
            g += 1
        blocks.append((col, g, Lb, i))
        col += g * Lb
        i += g
    ipad = col
    slot_col = np.empty(nslots, np.int64)
    for (c0, g, Lb, s0) in blocks:
        for q in range(g):
            slot_col[s0 + q] = c0 + q * Lb
    return order, blocks, ipad, slot_col


def _prep(inputs):
    cat = np.asarray(inputs["category_idx"]).astype(np.int64).ravel()
    cat_sizes = np.bincount(cat, minlength=NUM_CATS)
    order, blocks, ipad, slot_col = _layout(cat_sizes)

    rank = np.full(NUM_CATS, -1, np.int64)
    rank[order] = np.arange(len(order))

    perm = np.argsort(cat, kind="stable")
    starts = np.searchsorted(cat[perm], np.arange(NUM_CATS))
    within_sorted = np.arange(NUM_ITEMS) - starts[cat[perm]]
    item_within = np.empty(NUM_ITEMS, np.int64)
    item_within[perm] = within_sorted

    r = rank[cat]
    item_shard = r % NCORES
    item_col = slot_col[r // NCORES] + item_within

    alpha = np.ascontiguousarray(np.asarray(inputs["alpha_item"], np.float32))
    obs = np.ascontiguousarray(np.asarray(inputs["item_obs"], np.float32))
    lam = np.asarray(inputs["lambda_item"], np.float32).ravel()

    W = np.zeros((NCORES, 2 * LATENT, ipad), np.float32)
    LAMS = np.full((NCORES, 1, ipad), PAD_NEG, np.float32)
    for s in range(NCORES):
        m = item_shard == s
        cols = item_col[m]
        W[s, 0:LATENT, cols] = alpha[m]
        W[s, LATENT:, cols] = obs[m]
        LAMS[s, 0, cols] = lam[m] - SHIFT
    W = W.astype(ml_dtypes.bfloat16)
    LAMS = LAMS.astype(ml_dtypes.bfloat16)

    uidx = np.asarray(inputs["user_index"]).astype(np.int64).ravel()
    theta = np.asarray(inputs["theta_user"], np.float32)
    zeta = np.asarray(inputs["zeta_user"], np.float32)
    thzet = np.ascontiguousarray(
        np.concatenate([theta[uidx], zeta[uidx]], axis=1).T
    ).astype(ml_dtypes.bfloat16)
    return {
        "blocks": blocks,
        "ipad": ipad,
        "item_shard": item_shard,
        "item_col": item_col,
        "W": W,
        "LAMS": LAMS,
        "thzet": thzet,
    }


# ----------------------------------------------------------------------------
# Device program
# ----------------------------------------------------------------------------

def _bcast3(t2d, goff, g, L):
    """[P, G] tile slice [:, goff:goff+g] -> [P, g, L] step-0 broadcast AP."""
    ap = t2d[:, goff:goff + g]
    return bass.AP(tensor=ap.tensor, offset=ap.offset, ap=[*ap.ap, [0, L]])


def _view3(t2d, off, g, stride, w):
    """[P, N] tile -> [P, g, w] AP starting at column `off`, slot stride
    `stride` elements, innermost width w step-1."""
    ap = t2d[:, :]
    return bass.AP(
        tensor=ap.tensor, offset=ap.offset + off,
        ap=[ap.ap[0], [stride, g], [1, w]],
    )


def _build_nc(blocks, ipad):
    nc = bacc.Bacc(
        "TRN2",
        debug=False,
        enable_asserts=False,
        target_bir_lowering=False,
        num_devices=NCORES,
    )
    w_d = nc.dram_tensor("W", [2 * LATENT, ipad], BF16, kind="ExternalInput").ap()
    lams_d = nc.dram_tensor("LAMS", [1, ipad], BF16, kind="ExternalInput").ap()
    thzet_d = nc.dram_tensor("THZET", [2 * LATENT, BATCH], BF16, kind="ExternalInput").ap()
    out_d = nc.dram_tensor("O", [BATCH, ipad], BF16, kind="ExternalOutput").ap()

    gtot = sum(g for (_c, g, _l, _s) in blocks)
    with tile.TileContext(nc) as tc:
        with (
            tc.tile_pool(name="singles", bufs=1) as singles,
            tc.tile_pool(name="psum_u", bufs=4, space="PSUM") as psum_u,
            tc.tile_pool(name="exbuf", bufs=8) as exbuf,
            tc.tile_pool(name="treebuf", bufs=4) as treebuf,
            tc.tile_pool(name="stats", bufs=6) as stats,
            tc.tile_pool(name="obuf", bufs=4) as obuf,
        ):
            thzet_sb = singles.tile([2 * LATENT, BATCH], BF16, name="thzet_sb")
            nc.sync.dma_start(out=thzet_sb[:, :], in_=thzet_d[:, :])
            ones_sb = singles.tile([1, P], BF16, name="ones_sb")
            nc.vector.memset(ones_sb[:, :], 1.0)
            thze_t = [thzet_sb[:, j * P:(j + 1) * P] for j in range(NCHUNKS)]
            w_sb = singles.tile([2 * LATENT, ipad], BF16, name="w_sb")
            lams_sb = singles.tile([1, ipad], BF16, name="lams_sb")
            for (col0, g, L, _s0) in blocks:
                cols = g * L
                # scalar HWDGE ring: streams in parallel with thzet on sync
                nc.scalar.dma_start(
                    out=w_sb[:, col0:col0 + cols], in_=w_d[:, col0:col0 + cols]
                )
                nc.scalar.dma_start(
                    out=lams_sb[:, col0:col0 + cols],
                    in_=lams_d[:, col0:col0 + cols],
                )

            goffs = []
            acc = 0
            for (_c, g, _l, _s) in blocks:
                goffs.append(acc)
                acc += g
            for j in range(NCHUNKS):
                s_g = stats.tile([P, gtot], F32, name="s_g", tag="s_g")
                out_sb = obuf.tile([P, ipad], BF16, name="out_sb", tag="out_sb")
                border = list(range(len(blocks)))
                if j == NCHUNKS - 1 and len(blocks) > 1:
                    # drain: end the last chunk on the tiniest block so the
                    # final tree->lsc->final->DMA tail chain is short
                    border = border[1:] + [border[0]]
                deferred = []
                for bi in border:
                    col0, g, L, _s0 = blocks[bi]
                    goff = goffs[bi]
                    cols = g * L
                    ex = exbuf.tile([P, cols], BF16, name="ex", tag="ex")
                    # PSUM in 1024-col pieces (ring of 4) so the PE stream
                    # stays dense; per piece: both util halves, then both lam
                    # halves (one LDWEIGHTS switch pair), then one exp evict
                    for c0 in range(0, cols, 1024):
                        cn = min(1024, cols - c0)
                        up = psum_u.tile([P, 1024], F32, name="up", tag="up")
                        for d0 in range(0, cn, 512):
                            dn = min(512, cn - d0)
                            nc.tensor.matmul(
                                up[:, d0:d0 + dn],
                                lhsT=thze_t[j],
                                rhs=w_sb[:, col0 + c0 + d0:col0 + c0 + d0 + dn],
                                start=True,
                                stop=False,
                            )
                        for d0 in range(0, cn, 512):
                            dn = min(512, cn - d0)
                            nc.tensor.matmul(
                                up[:, d0:d0 + dn],
                                lhsT=ones_sb[0:1, :],
                                rhs=lams_sb[0:1, col0 + c0 + d0:col0 + c0 + d0 + dn],
                                start=False,
                                stop=True,
                            )
                        nc.scalar.activation(
                            out=ex[:, c0:c0 + cn], in_=up[:, 0:cn],
                            func=mybir.ActivationFunctionType.Exp,
                        )
                    # segment sums: halving tree in bf16 (2x), then reduce
                    w_ = L
                    cur = ex
                    stride = L
                    while g > 1 and w_ % 2 == 0 and w_ > 18:
                        h = w_ // 2
                        nxt = treebuf.tile([P, g * h], BF16, name="tr", tag="tr")
                        nc.vector.tensor_add(
                            out=_view3(nxt, 0, g, h, h),
                            in0=_view3(cur, 0, g, stride, h),
                            in1=_view3(cur, h, g, stride, h),
                        )
                        cur, stride, w_ = nxt, h, h
                    nc.vector.tensor_reduce(
                        out=s_g[:, goff:goff + g],
                        in_=_view3(cur, 0, g, stride, w_),
                        axis=mybir.AxisListType.X,
                        op=mybir.AluOpType.add,
                    )
                    # per-block epilogue right away: lsc for this block's
                    # slots, fused final, out-DMA. Keeps VectorE fed while
                    # the next block's exp is still cooking and shortens the
                    # last-chunk tail chain.
                    # lsc = bitcast_i32(s) * ln2/2^23  (the +(127+sig)*ln2
                    # bias cancels against the identical bias in the bf16
                    # hack of the final)
                    lscb = stats.tile([P, g], F32, name="lscb", tag="lscb")
                    nc.vector.tensor_scalar(
                        out=lscb[:, :],
                        in0=s_g[:, goff:goff + g].bitcast(I32),
                        scalar1=K32, scalar2=None,
                        op0=mybir.AluOpType.mult,
                    )
                    if g * L >= 1500 and j < NCHUNKS - 1:
                        # big block: ScalarE expands lsc to a dense bf16 row;
                        # V then runs lnx at 4x and the subtract as an
                        # all-bf16 step-1 TT at 2x (STT has no 2x uop).
                        # Deferred to chunk end so the S-side lsx never
                        # head-of-line blocks the remaining exps.
                        lnx = treebuf.tile([P, g * L], BF16, name="lnx", tag="lnx")
                        nc.vector.tensor_scalar(
                            out=lnx[:, :], in0=ex[:, 0:g * L].bitcast(I16),
                            scalar1=K16, scalar2=None,
                            op0=mybir.AluOpType.mult,
                        )
                        deferred.append((col0, g, L, lscb, lnx))
                    else:
                        nc.vector.scalar_tensor_tensor(
                            out=_view3(out_sb, col0, g, L, L),
                            in0=_view3(ex, 0, g, L, L).bitcast(I16),
                            scalar=K16,
                            in1=_bcast3(lscb, 0, g, L),
                            op0=mybir.AluOpType.mult,
                            op1=mybir.AluOpType.subtract,
                        )
                        nc.sync.dma_start(
                            out=out_d[j * P:(j + 1) * P, col0:col0 + g * L],
                            in_=out_sb[:, col0:col0 + g * L],
                        )
                if deferred:
                    for (col0, g, L, lscb, lnx) in deferred:
                        lsx = treebuf.tile([P, g * L], BF16, name="lsx", tag="lsx")
                        nc.scalar.copy(
                            out=lsx[:, :], in_=_bcast3(lscb, 0, g, L)
                        )
                        nc.vector.tensor_tensor(
                            out=out_sb[:, col0:col0 + g * L],
                            in0=lnx[:, :],
                            in1=lsx[:, :],
                            op=mybir.AluOpType.subtract,
                        )
                        nc.sync.dma_start(
                            out=out_d[j * P:(j + 1) * P, col0:col0 + g * L],
                            in_=out_sb[:, col0:col0 + g * L],
                        )
    nc.compile()
    return nc


# ----------------------------------------------------------------------------
# Entry points
# ----------------------------------------------------------------------------

def run(inputs, trace=False):
    prep = _prep(inputs)
    key = (prep["ipad"], tuple(prep["blocks"]))
    nc = _nc_cache.get(key)
    if nc is None:
        print(f"[kernel] ipad={prep['ipad']} blocks={prep['blocks']}",
              file=sys.stderr)
        nc = _build_nc(prep["blocks"], prep["ipad"])
        _nc_cache[key] = nc
    in_maps = [
        {
            "W": prep["W"][c],
            "LAMS": prep["LAMS"][c],
            "THZET": prep["thzet"],
        }
        for c in range(NCORES)
    ]
    res = bass_utils.run_bass_kernel_spmd(
        nc, in_maps, core_ids=list(range(NCORES)), trace=trace
    )
    big = np.stack(
        [np.asarray(res.results[c]["O"]).astype(np.float32) for c in range(NCORES)]
    )  # [8, B, ipad] f32
    out = np.ascontiguousarray(
        big[prep["item_shard"], :, prep["item_col"]].T
    ).astype(np.float32)
    return out, res


def kernel(**inputs) -> np.ndarray:
    out, _ = run(inputs, trace=False)
    return out


# revision 39
# speedup vs baseline: 1.0987x; 1.0987x over previous
"""Trainium2 Bass kernel for nn_BEMBFlex (within-category log-softmax utility model).

Sharding: items dealt by category across the 8 cores (categories rank-sorted
by size, rank % 8 -> shard), so one SPMD program serves all cores. Each core
computes util for all 1024 sessions over its ~1/8 of the items, then the
within-category log-softmax locally (categories never span shards).

v2 pipeline (per 128-session chunk, per column block):
  TensorE:  psum = [th|ze] @ [alphaT; item_obsT]  (+ rank-1 matmul folding
            the per-item (lambda - SHIFT) row, so PSUM holds u1 directly);
            512-col PSUM pieces in a ring of 4x1024 keep the PE stream dense
  ScalarE:  ex_bf16 = Exp(psum)            (the only PSUM reader; 1 ACT pass)
  VectorE:  segment sums via bf16 halving tree (2x mode) + tensor_reduce
  VectorE:  lsc = bitcast_i32(s) * ln2/2^23       (log2 bit-hack, no ACT Ln)
  VectorE:  out_bf16 = (bitcast_i16(ex) * ln2/2^7) - lsc_bcast  (fused STT:
            ln(ex) via the same bit-hack; the (127+sigma)*ln2 biases cancel
            between the two hacks). GpSimd deliberately unused: it shares an
            SBUF port with the DVE, and offloading finals there slowed the
            DVE 2-port ops ~3x (measured).
  DMA out per block (bf16, largest blocks first), host de-permutes + casts.
"""

import sys

for _p in ("/opt/trn_rl_repo",):
    if _p not in sys.path:
        sys.path.insert(0, _p)

import ml_dtypes
import numpy as np

import concourse.bass as bass
import concourse.tile as tile
from concourse import bacc, bass_utils, mybir

NUM_USERS = 100000
NUM_ITEMS = 25000
NUM_CATS = 500
LATENT = 64
BATCH = 1024
NCORES = 8
P = 128
NCHUNKS = BATCH // P
BLOCK_COLS = 2048
PAD_NEG = -1.0e30
SHIFT = 18.0
LN2 = float(np.log(2.0))
K16 = LN2 / (1 << 7)       # bf16-bits  -> ln scale
K32 = LN2 / (1 << 23)      # fp32-bits  -> ln scale

F32 = mybir.dt.float32
BF16 = mybir.dt.bfloat16
I16 = mybir.dt.int16
I32 = mybir.dt.int32

_nc_cache = {}


# ----------------------------------------------------------------------------
# Host-side layout
# ----------------------------------------------------------------------------

def _layout(cat_sizes):
    """Slot/block layout shared by all 8 shards.

    Categories sorted by size desc; slot i holds category ranks [8i, 8i+8)
    (one per shard). Slot width L_i = largest size in the group rounded up
    to a multiple of 8 (so the halving tree stays 2x-mode friendly). Blocks
    greedily group consecutive slots under a uniform L.
    """
    order = np.argsort(-cat_sizes, kind="stable")
    order = order[cat_sizes[order] > 0]
    ncats = len(order)
    nslots = -(-ncats // NCORES)
    slot_L = np.empty(nslots, np.int64)
    for i in range(nslots):
        mx = int(cat_sizes[order[i * NCORES]])
        slot_L[i] = max(8, ((mx + 7) // 8) * 8)
    blocks = []  # (col0, g, L, slot0)
    col = 0
    i = 0
    while i < nslots:
        Lb = int(slot_L[i])
        g = 1
        sm = Lb
        # grow while under the col cap AND padding waste stays under 10%
        while (
            i + g < nslots
            and (g + 1) * Lb <= BLOCK_COLS
            and (g + 1) * Lb - (sm + slot_L[i + g]) <= 0.10 * (g + 1) * Lb
        ):
            sm += slot_L[i + g]
            g += 1
        blocks.append((col, g, Lb, i))
        col += g * Lb
        i += g
    ipad = col
    slot_col = np.empty(nslots, np.int64)
    for (c0, g, Lb, s0) in blocks:
        for q in range(g):
            slot_col[s0 + q] = c0 + q * Lb
    return order, blocks, ipad, slot_col


def _prep(inputs):
    cat = np.asarray(inputs["category_idx"]).astype(np.int64).ravel()
    cat_sizes = np.bincount(cat, minlength=NUM_CATS)
    order, blocks, ipad, slot_col = _layout(cat_sizes)

    rank = np.full(NUM_CATS, -1, np.int64)
    rank[order] = np.arange(len(order))

    perm = np.argsort(cat, kind="stable")
    starts = np.searchsorted(cat[perm], np.arange(NUM_CATS))
    within_sorted = np.arange(NUM_ITEMS) - starts[cat[perm]]
    item_within = np.empty(NUM_ITEMS, np.int64)
    item_within[perm] = within_sorted

    r = rank[cat]
    item_shard = r % NCORES
    item_col = slot_col[r // NCORES] + item_within

    alpha = np.ascontiguousarray(np.asarray(inputs["alpha_item"], np.float32))
    obs = np.ascontiguousarray(np.asarray(inputs["item_obs"], np.float32))
    lam = np.asarray(inputs["lambda_item"], np.float32).ravel()

    W = np.zeros((NCORES, 2 * LATENT, ipad), np.float32)
    LAMS = np.full((NCORES, 1, ipad), PAD_NEG, np.float32)
    for s in range(NCORES):
        m = item_shard == s
        cols = item_col[m]
        W[s, 0:LATENT, cols] = alpha[m]
        W[s, LATENT:, cols] = obs[m]
        LAMS[s, 0, cols] = lam[m] - SHIFT
    W = W.astype(ml_dtypes.bfloat16)
    LAMS = LAMS.astype(ml_dtypes.bfloat16)

    uidx = np.asarray(inputs["user_index"]).astype(np.int64).ravel()
    theta = np.asarray(inputs["theta_user"], np.float32)
    zeta = np.asarray(inputs["zeta_user"], np.float32)
    thzet = np.ascontiguousarray(
        np.concatenate([theta[uidx], zeta[uidx]], axis=1).T
    ).astype(ml_dtypes.bfloat16)
    return {
        "blocks": blocks,
        "ipad": ipad,
        "item_shard": item_shard,
        "item_col": item_col,
        "W": W,
        "LAMS": LAMS,
        "thzet": thzet,
    }


# ----------------------------------------------------------------------------
# Device program
# ----------------------------------------------------------------------------

def _bcast3(t2d, goff, g, L):
    """[P, G] tile slice [:, goff:goff+g] -> [P, g, L] step-0 broadcast AP."""
    ap = t2d[:, goff:goff + g]
    return bass.AP(tensor=ap.tensor, offset=ap.offset, ap=[*ap.ap, [0, L]])


def _view3(t2d, off, g, stride, w):
    """[P, N] tile -> [P, g, w] AP starting at column `off`, slot stride
    `stride` elements, innermost width w step-1."""
    ap = t2d[:, :]
    return bass.AP(
        tensor=ap.tensor, offset=ap.offset + off,
        ap=[ap.ap[0], [stride, g], [1, w]],
    )


def _build_nc(blocks, ipad):
    nc = bacc.Bacc(
        "TRN2",
        debug=False,
        enable_asserts=False,
        target_bir_lowering=False,
        num_devices=NCORES,
    )
    w_d = nc.dram_tensor("W", [2 * LATENT, ipad], BF16, kind="ExternalInput").ap()
    lams_d = nc.dram_tensor("LAMS", [1, ipad], BF16, kind="ExternalInput").ap()
    thzet_d = nc.dram_tensor("THZET", [2 * LATENT, BATCH], BF16, kind="ExternalInput").ap()
    out_d = nc.dram_tensor("O", [BATCH, ipad], BF16, kind="ExternalOutput").ap()

    gtot = sum(g for (_c, g, _l, _s) in blocks)
    with tile.TileContext(nc) as tc:
        with (
            tc.tile_pool(name="singles", bufs=1) as singles,
            tc.tile_pool(name="psum_u", bufs=4, space="PSUM") as psum_u,
            tc.tile_pool(name="exbuf", bufs=8) as exbuf,
            tc.tile_pool(name="treebuf", bufs=4) as treebuf,
            tc.tile_pool(name="stats", bufs=6) as stats,
            tc.tile_pool(name="obuf", bufs=4) as obuf,
        ):
            thzet_sb = singles.tile([2 * LATENT, BATCH], BF16, name="thzet_sb")
            nc.sync.dma_start(out=thzet_sb[:, :], in_=thzet_d[:, :])
            ones_sb = singles.tile([1, P], BF16, name="ones_sb")
            nc.vector.memset(ones_sb[:, :], 1.0)
            thze_t = [thzet_sb[:, j * P:(j + 1) * P] for j in range(NCHUNKS)]
            w_sb = singles.tile([2 * LATENT, ipad], BF16, name="w_sb")
            lams_sb = singles.tile([1, ipad], BF16, name="lams_sb")
            for (col0, g, L, _s0) in blocks:
                cols = g * L
                # scalar HWDGE ring: streams in parallel with thzet on sync
                nc.scalar.dma_start(
                    out=w_sb[:, col0:col0 + cols], in_=w_d[:, col0:col0 + cols]
                )
                nc.scalar.dma_start(
                    out=lams_sb[:, col0:col0 + cols],
                    in_=lams_d[:, col0:col0 + cols],
                )

            goffs = []
            acc = 0
            for (_c, g, _l, _s) in blocks:
                goffs.append(acc)
                acc += g
            for j in range(NCHUNKS):
                s_g = stats.tile([P, gtot], F32, name="s_g", tag="s_g")
                out_sb = obuf.tile([P, ipad], BF16, name="out_sb", tag="out_sb")
                border = list(range(len(blocks)))
                if j == NCHUNKS - 1 and len(blocks) > 1:
                    # drain: end the last chunk on the tiniest block so the
                    # final tree->lsc->final->DMA tail chain is short
                    border = border[1:] + [border[0]]
                for bi in border:
                    col0, g, L, _s0 = blocks[bi]
                    goff = goffs[bi]
                    cols = g * L
                    ex = exbuf.tile([P, cols], BF16, name="ex", tag="ex")
                    # PSUM in 1024-col pieces (ring of 4) so the PE stream
                    # stays dense; per piece: both util halves, then both lam
                    # halves (one LDWEIGHTS switch pair), then one exp evict
                    for c0 in range(0, cols, 1024):
                        cn = min(1024, cols - c0)
                        up = psum_u.tile([P, 1024], F32, name="up", tag="up")
                        for d0 in range(0, cn, 512):
                            dn = min(512, cn - d0)
                            nc.tensor.matmul(
                                up[:, d0:d0 + dn],
                                lhsT=thze_t[j],
                                rhs=w_sb[:, col0 + c0 + d0:col0 + c0 + d0 + dn],
                                start=True,
                                stop=False,
                            )
                        for d0 in range(0, cn, 512):
                            dn = min(512, cn - d0)
                            nc.tensor.matmul(
                                up[:, d0:d0 + dn],
                                lhsT=ones_sb[0:1, :],
                                rhs=lams_sb[0:1, col0 + c0 + d0:col0 + c0 + d0 + dn],
                                start=False,
                                stop=True,
                            )
                        nc.scalar.activation(
                            out=ex[:, c0:c0 + cn], in_=up[:, 0:cn],
                            func=mybir.ActivationFunctionType.Exp,
                        )
                    # segment sums: halving tree in bf16 (2x), then reduce
                    w_ = L
                    cur = ex
                    stride = L
                    while g > 1 and w_ % 2 == 0 and w_ > 18:
                        h = w_ // 2
                        nxt = treebuf.tile([P, g * h], BF16, name="tr", tag="tr")
                        nc.vector.tensor_add(
                            out=_view3(nxt, 0, g, h, h),
                            in0=_view3(cur, 0, g, stride, h),
                            in1=_view3(cur, h, g, stride, h),
                        )
                        cur, stride, w_ = nxt, h, h
                    nc.vector.tensor_reduce(
                        out=s_g[:, goff:goff + g],
                        in_=_view3(cur, 0, g, stride, w_),
                        axis=mybir.AxisListType.X,
                        op=mybir.AluOpType.add,
                    )
                    # per-block epilogue right away: lsc for this block's
                    # slots, fused final, out-DMA. Keeps VectorE fed while
                    # the next block's exp is still cooking and shortens the
                    # last-chunk tail chain.
                    # lsc = bitcast_i32(s) * ln2/2^23  (the +(127+sig)*ln2
                    # bias cancels against the identical bias in the bf16
                    # hack of the final)
                    lscb = stats.tile([P, g], F32, name="lscb", tag="lscb")
                    nc.vector.tensor_scalar(
                        out=lscb[:, :],
                        in0=s_g[:, goff:goff + g].bitcast(I32),
                        scalar1=K32, scalar2=None,
                        op0=mybir.AluOpType.mult,
                    )
                    nc.vector.scalar_tensor_tensor(
                        out=_view3(out_sb, col0, g, L, L),
                        in0=_view3(ex, 0, g, L, L).bitcast(I16),
                        scalar=K16,
                        in1=_bcast3(lscb, 0, g, L),
                        op0=mybir.AluOpType.mult,
                        op1=mybir.AluOpType.subtract,
                    )
                    nc.sync.dma_start(
                        out=out_d[j * P:(j + 1) * P, col0:col0 + g * L],
                        in_=out_sb[:, col0:col0 + g * L],
                    )
    nc.compile()
    return nc


# ----------------------------------------------------------------------------
# Entry points
# ----------------------------------------------------------------------------

def run(inputs, trace=False):
    prep = _prep(inputs)
    key = (prep["ipad"], tuple(prep["blocks"]))
    nc = _nc_cache.get(key)
    if nc is None:
        print(f"[kernel] ipad={prep['ipad']} blocks={prep['blocks']}",
              file=sys.stderr)
        nc = _build_nc(prep["blocks"], prep["ipad"])
        _nc_cache[key] = nc
    in_maps = [
        {
            "W": prep["W"][c],
            "LAMS": prep["LAMS"][c],
            "THZET": prep["thzet"],
        }
        for c in range(NCORES)
    ]
    res = bass_utils.run_bass_kernel_spmd(
        nc, in_maps, core_ids=list(range(NCORES)), trace=trace
    )
    big = np.stack(
        [np.asarray(res.results[c]["O"]).astype(np.float32) for c in range(NCORES)]
    )  # [8, B, ipad] f32
    out = np.ascontiguousarray(
        big[prep["item_shard"], :, prep["item_col"]].T
    ).astype(np.float32)
    return out, res


def kernel(**inputs) -> np.ndarray:
    out, _ = run(inputs, trace=False)
    return out
